# revision 15
# baseline (speedup 1.0000x reference)
"""Exact entmax-1.5 loss kernel for Trainium2 (8 NeuronCores, data-parallel over rows).

Algorithm (per row of X [N=2048, V=32000] f32):
  The entmax-1.5 threshold tau* solves  sum_j relu(X_j/2 - tau)^2 = 1.
  In X-units (theta = 2*tau):            sum_j relu(X_j - theta)^2 = 4.
  f(theta) is convex decreasing; Newton from a lower bound converges
  monotonically from below - no sort needed.

  v4 pipeline per 128-row block (fp16 payloads / bf16 bounds):
    A. Stream X in 16 f32 column chunks (DMA from SP queue); scalar converts
       to resident fp16 xh; vector builds group-of-4 maxes g4 bf16 and
       (per 4-chunk quarter) group-of-64 maxes g64 bf16.  A 4-iteration
       partial Newton on the first 3 quarters of g64 warm-starts theta while
       the last quarter still streams.
    B. 3 full all-vector Newton iterations on g64 -> theta_lb.
    C. mask = relu(g4 - theta_lb) in place; 16 accumulating 500-col bf16
       matmuls with residue-selection weights -> per-cluster group activity
       in wrapped [128,500] PSUM layout.
    D. Compaction: iota*mask, scan -> ranks, local_scatter -> 64 gids per
       partition, sentinel fixup (f16 mask/cum scratch).
    E. Two ap_gathers (512 idxs each) pull 4-fp16 payloads of the
       cluster-union candidate groups from xh -> cand [128,4096] fp16.
    F. Exact Newton (2 iters) + final stats on cand, column-split between
       scalar (activation accumulators) and vector engines;
       loss = 4/3 + S3/12 + theta*S2/4 - X[target] (X[target] via up-front
       dma_gather + one-hot dot, f32 exact).

  Emission order software-pipelines the two blocks: A0, B0..E0, A1, F0,
  B1..E1, F1 so block 1 streams under block 0's tail and the two ~30us
  gpsimd gathers overlap other engines' work.

Host wrapper shards rows 256-per-core across 8 cores, no collectives.
"""
import numpy as np
from contextlib import ExitStack

N, V = 2048, 32000
N_CORES = 8
ROWS = N // N_CORES          # 256 rows per core
CHUNK = 1600
NCHUNK = V // CHUNK          # 20
NG4 = V // 4                 # 8000 groups of 4
WF = 500                     # wrapped cols: group = 16*F + (p % 16)
PART_ITERS = 4               # partial-newton iters on first 375 g64 cols
FULL_ITERS = 3
EXACT_ITERS = 2
EPS_LB = 2e-2                # X-units safety margin (covers bf16 bound noise)
S4 = 60                      # per-partition capacity of compacted group ids
KU = 16 * S4                 # 1024 union groups per 16-partition cluster
CW = 4 * KU                  # 4096 compact width (fp16)
GPAD = 4
NGP = NG4 + GPAD             # 8008 groups incl. sentinel pad
DW = 4 * NGP                 # 32032 fp16 X width
SENT = NG4 + 2               # sentinel group id for scatter pads
HF = 2560                    # F-phase split: scalar [0:HF], vector [HF:CW]
NEG = -60000.0               # fp16-safe -inf substitute

_nc_cache = {}


def _build_nc():
    import concourse.bass as bass
    import concourse.bacc as bacc
    import concourse.tile as tile
    from concourse import mybir

    f32 = mybir.dt.float32
    f16 = mybir.dt.float16
    bf16 = mybir.dt.bfloat16
    i16 = mybir.dt.int16
    Alu = mybir.AluOpType
    Act = mybir.ActivationFunctionType
    Ax = mybir.AxisListType

    nc = bacc.Bacc("TRN2", target_bir_lowering=False, debug=False)
    x = nc.dram_tensor("x", [ROWS, V], f32, kind="ExternalInput").ap()
    oh = nc.dram_tensor("oh", [ROWS, 64], f32, kind="ExternalInput").ap()
    tbl = nc.dram_tensor("tbl", [128, 16], i16, kind="ExternalInput").ap()
    iotd = nc.dram_tensor("iot", [128, WF], f32, kind="ExternalInput").ap()
    wseld = nc.dram_tensor("wsel", [128, 16 * 128], bf16, kind="ExternalInput").ap()
    out = nc.dram_tensor("loss", [ROWS], f32, kind="ExternalOutput").ap()

    with tile.TileContext(nc) as tc, ExitStack() as ctx:
        const = ctx.enter_context(tc.tile_pool(name="const", bufs=1))
        big = ctx.enter_context(tc.tile_pool(name="big", bufs=1))
        psum = ctx.enter_context(tc.tile_pool(name="psum", bufs=2, space="PSUM"))

        iot = const.tile([128, WF], f32, tag="iot")
        wt = const.tile([128, 16 * 128], bf16, tag="wsel")
        xtg = const.tile([128, 2], f32, tag="xtg")
        tbl_t = const.tile([128, 16], i16, tag="tbl")

        nc.sync.dma_start(iot[:], iotd)
        nc.sync.dma_start(wt[:], wseld)
        nc.sync.dma_start(tbl_t[:], tbl[:, :])

        # ---- x[target] DMA gathers for both blocks, up front (gpsimd mlp lib).
        # The one-hot dot products are deferred to phase F to keep the vector
        # queue free for streaming.
        gtiles = []
        for b in range(2):
            gA = big.tile([128, 64], f32, tag="gA", bufs=2)
            gB = big.tile([128, 64], f32, tag="gB", bufs=2)
            for half, gdst in ((0, gA), (1, gB)):
                c = 2 * b + half
                src = x[64 * c : 64 * (c + 1), :].rearrange(
                    "r (bk e) -> (r bk) e", e=64
                )
                nc.gpsimd.dma_gather(
                    gdst.rearrange("p (one e) -> p one e", one=1),
                    src,
                    tbl_t[:, 4 * c : 4 * (c + 1)],
                    num_idxs=64,
                    num_idxs_reg=64,
                    elem_size=64,
                )
            nc.gpsimd.dma_start(gA[64:128, :], gB[0:64, :])
            # after the combine, gB is reloaded with the one-hot rows; issued
            # from the gpsimd queue so the dependency cannot block sync's
            # in-order chunk-trigger stream
            nc.gpsimd.dma_start(gB[:], oh[bass.ts(b, 128), :])
            gtiles.append((gA, gB))

        # dummy 16-idx ap_gather: pre-loads the Q7 ap_gather library while
        # block 0 is still streaming, so the real gathers start instantly
        sc0_w = big.tile([128, 16], f32, tag="warm", bufs=1)
        wix = sc0_w[:, 15:16].bitcast(i16)[:, 0:1]
        nc.vector.memset(wix, 0)
        nc.gpsimd.ap_gather(
            sc0_w[:, 0:8].rearrange("p (a d) -> p a d", d=4),
            sc0_w[:, 8:12].rearrange("p (a d) -> p a d", d=4),
            wix,
            channels=128,
            num_elems=4,
            d=4,
            num_idxs=16,
        )

        # per-block tiles (tag rotation gives block-alternating buffers)
        tiles = []
        for b in range(2):
            t = {}
            t["xh"] = big.tile([128, DW], f16, name="xh", tag="xh", bufs=2)
            t["g4"] = big.tile([128, NG4], bf16, name="g4", tag="g4", bufs=2)
            t["cand"] = big.tile([128, CW], f16, name="cand", tag="cand", bufs=2)
            t["wk"] = big.tile([128, HF], f16, name="wk", tag="wk", bufs=1)
            t["sc"] = big.tile([128, 128], f32, name="sc", tag="sc", bufs=2)
            t["scr"] = big.tile([128, 768], f32, name="scr", tag="scr", bufs=2)
            tiles.append(t)

        def g64_of(t):
            return t["scr"][:, 512:768].bitcast(bf16)

        def newton_iters(t, g, width, iters):
            sc, scr = t["sc"], t["scr"]
            th = sc[:, 1:2]
            nuB = sc[:, 2:3]
            S1a = sc[:, 3:4]
            S2a = sc[:, 4:5]
            r1 = sc[:, 7:8]
            dd = sc[:, 8:9]
            uB = scr[:, 0:256].bitcast(bf16)
            for _ in range(iters):
                nc.vector.tensor_scalar(out=nuB, in0=th, scalar1=-1.0, scalar2=None, op0=Alu.mult)
                nc.scalar.activation(
                    uB[:, 0:width], g[:, 0:width], Act.Relu, bias=nuB, scale=1.0,
                    accum_out=S1a,
                )
                nc.scalar.activation(uB[:, 0:width], uB[:, 0:width], Act.Square, accum_out=S2a)
                nc.vector.reciprocal(r1, S1a)
                nc.vector.tensor_scalar(
                    out=dd, in0=S2a, scalar1=-4.0, scalar2=0.5, op0=Alu.add, op1=Alu.mult
                )
                nc.vector.scalar_tensor_tensor(
                    out=th, in0=dd, scalar=r1, in1=th, op0=Alu.mult, op1=Alu.add
                )

        def phase_A(b):
            t = tiles[b]
            xh, g4, sc = t["xh"], t["g4"], t["sc"]
            g64 = g64_of(t)
            m_s = sc[:, 0:1]
            th = sc[:, 1:2]
            nc.vector.memset(xh[:, V:DW], NEG)
            xb = x[bass.ts(b, 128), :]
            for c in range(NCHUNK):
                xc = big.tile([128, CHUNK], f32, tag="xc", bufs=2)
                nc.sync.dma_start(xc[:], xb[:, bass.ts(c, CHUNK)])
                nc.scalar.activation(
                    xh[:, bass.ts(c, CHUNK)], xc[:], Act.Copy, bias=0.0, scale=1.0
                )
                g4p = g4[:].rearrange("p (w F) -> p F w", w=16)
                nc.vector.tensor_reduce(
                    g4p[:, 25 * c : 25 * (c + 1), :],
                    xh[:, bass.ts(c, CHUNK)].rearrange(
                        "p (f w k) -> p f w k", w=16, k=4
                    ),
                    axis=Ax.X,
                    op=Alu.max,
                )
                if c % 5 == 4:
                    q = c // 5
                    nc.vector.tensor_reduce(
                        g64[:, bass.ts(q, 125)],
                        g4p[:, 125 * q : 125 * (q + 1), :],
                        axis=Ax.X,
                        op=Alu.max,
                    )
                if c == 14:
                    # partial-newton warm start on quarters 0-2 (375 cols)
                    nc.vector.tensor_reduce(m_s, g64[:, 0:375], axis=Ax.X, op=Alu.max)
                    nc.vector.tensor_scalar(
                        out=th, in0=m_s, scalar1=-2.0, scalar2=None, op0=Alu.add
                    )
                    newton_iters(t, g64, 375, PART_ITERS)

        def phase_BCDE(b):
            t = tiles[b]
            xh, g4, sc, scr = t["xh"], t["g4"], t["sc"], t["scr"]
            g64 = g64_of(t)

            th = sc[:, 1:2]
            nu = sc[:, 2:3]
            vcomp = sc[:, 16:48].bitcast(i16)      # [128, 64] i16
            bneg = sc[:, 48:112]                    # [128, 64] f32

            maskv = scr[:, 0:250].bitcast(f16)      # [128, 500] f16
            cum = scr[:, 250:500].bitcast(f16)      # [128, 500] f16
            v16 = scr[:, 512:768].bitcast(i16)      # [128, 512] i16
            rank = scr[:, 0:256].bitcast(i16)       # [128, 512] i16, reuses maskv

            # ---- B: finish the G2 newton on the full 500 cols ----
            newton_iters(t, g64, WF, FULL_ITERS)
            nc.vector.tensor_scalar(out=th, in0=th, scalar1=-EPS_LB, scalar2=None, op0=Alu.add)
            nc.vector.tensor_scalar(out=nu, in0=th, scalar1=-1.0, scalar2=None, op0=Alu.mult)

            # ---- C: candidate mask (in place on g4) + residue matmuls ----
            nc.vector.tensor_scalar(
                out=g4[:], in0=g4[:], scalar1=th, scalar2=0.0,
                op0=Alu.subtract, op1=Alu.max,
            )
            pc = psum.tile([128, WF], f32, tag="pc")
            for w in range(16):
                nc.tensor.matmul(
                    pc[:],
                    wt[:, bass.ts(w, 128)],
                    g4[:, bass.ts(w, WF)],
                    start=(w == 0),
                    stop=(w == 15),
                )

            # ---- D: compaction ----
            nc.vector.tensor_scalar(out=maskv, in0=pc[:], scalar1=0.0, scalar2=None, op0=Alu.is_gt)
            nc.vector.scalar_tensor_tensor(
                out=v16[:, 0:WF], in0=maskv, scalar=1.0, in1=iot[:],
                op0=Alu.mult, op1=Alu.mult,
            )
            nc.vector.tensor_tensor_scan(
                out=cum, data0=maskv, data1=maskv, initial=0.0,
                op0=Alu.add, op1=Alu.bypass,
            )
            nc.vector.tensor_tensor(out=cum, in0=cum, in1=maskv, op=Alu.mult)
            nc.vector.scalar_tensor_tensor(
                out=cum, in0=cum, scalar=float(S4) + 0.5, in1=cum,
                op0=Alu.is_le, op1=Alu.mult,
            )
            nc.vector.tensor_scalar(out=rank[:, 0:WF], in0=cum, scalar1=-1.0, scalar2=None, op0=Alu.add)
            nc.gpsimd.local_scatter(
                vcomp[:, 0:S4],
                v16[:, 0:WF],
                rank[:, 0:WF],
                channels=128,
                num_elems=S4,
                num_idxs=WF,
            )
            # group idx = (gid+1) - 1; scatter pads (0) map to sentinel group
            nc.vector.tensor_scalar(
                out=bneg[:, 0:S4], in0=vcomp[:, 0:S4], scalar1=0.5, scalar2=float(SENT) + 1.0,
                op0=Alu.is_lt, op1=Alu.mult,
            )
            nc.vector.scalar_tensor_tensor(
                out=bneg[:, 0:S4], in0=vcomp[:, 0:S4], scalar=1.0, in1=bneg[:, 0:S4],
                op0=Alu.mult, op1=Alu.add,
            )
            gix = v16[:, 0:S4]
            nc.vector.tensor_scalar(out=gix, in0=bneg[:, 0:S4], scalar1=-1.0, scalar2=None, op0=Alu.add)

            # ---- E: gather candidate payloads from xh (two halves) ----
            cand = t["cand"]
            for lo_s, hi_s in ((0, HF // 64), (HF // 64, S4)):
                nc.gpsimd.ap_gather(
                    cand[:, 64 * lo_s : 64 * hi_s].rearrange("p (a d) -> p a d", d=4),
                    xh[:].rearrange("p (a d) -> p a d", d=4),
                    gix[:, lo_s:hi_s],
                    channels=128,
                    num_elems=DW // 4,
                    d=4,
                    num_idxs=16 * (hi_s - lo_s),
                )

        def phase_F(b):
            t = tiles[b]
            cand, wk, sc, scr = t["cand"], t["wk"], t["sc"], t["scr"]
            gA, gB = gtiles[b]
            wkv = scr[:, 0:768].bitcast(f16)       # [128, 1536] vector-half u
            th = sc[:, 1:2]
            nu = sc[:, 2:3]
            S1a = sc[:, 3:4]
            S2a = sc[:, 4:5]
            S1b = sc[:, 5:6]
            S2b = sc[:, 6:7]
            r1 = sc[:, 7:8]
            dd = sc[:, 8:9]
            S2fa = sc[:, 9:10]
            S2fb = sc[:, 10:11]
            S3fa = sc[:, 11:12]
            S3fb = sc[:, 12:13]
            ta = sc[:, 13:14]
            tb_ = sc[:, 14:15]
            lo = sc[:, 15:16]
            one_t = sc[:, 0:1]   # reuses m_s slot (dead after phase A)
            VH = CW - HF

            for _ in range(EXACT_ITERS):
                nc.scalar.activation(
                    wk[:], cand[:, 0:HF], Act.Relu, bias=nu, scale=1.0, accum_out=S1a
                )
                nc.scalar.activation(wk[:], wk[:], Act.Square, accum_out=S2a)
                nc.vector.tensor_scalar(
                    out=wkv[:, 0:VH], in0=cand[:, HF:CW], scalar1=th, scalar2=0.0,
                    op0=Alu.subtract, op1=Alu.max,
                )
                nc.vector.tensor_reduce(S1b, wkv[:, 0:VH], axis=Ax.X, op=Alu.add)
                nc.vector.tensor_tensor(out=wkv[:, 0:VH], in0=wkv[:, 0:VH], in1=wkv[:, 0:VH], op=Alu.mult)
                nc.vector.tensor_reduce(S2b, wkv[:, 0:VH], axis=Ax.X, op=Alu.add)
                nc.vector.tensor_tensor(out=S1a, in0=S1a, in1=S1b, op=Alu.add)
                nc.vector.tensor_tensor(out=S2a, in0=S2a, in1=S2b, op=Alu.add)
                nc.vector.reciprocal(r1, S1a)
                nc.vector.tensor_scalar(
                    out=dd, in0=S2a, scalar1=-4.0, scalar2=0.5, op0=Alu.add, op1=Alu.mult
                )
                nc.vector.scalar_tensor_tensor(
                    out=th, in0=dd, scalar=r1, in1=th, op0=Alu.mult, op1=Alu.add
                )
                nc.vector.tensor_scalar(out=nu, in0=th, scalar1=-1.0, scalar2=None, op0=Alu.mult)

            # x[target] one-hot dot; one_t (==1.0) depends on the newton chain so
            # the static scheduler cannot hoist these to the head of the queue
            nc.vector.tensor_scalar(
                out=one_t, in0=r1, scalar1=0.0, scalar2=1.0, op0=Alu.mult, op1=Alu.add
            )
            nc.vector.scalar_tensor_tensor(
                out=gB[:], in0=gA[:], scalar=one_t, in1=gB[:], op0=Alu.mult, op1=Alu.mult
            )
            nc.vector.tensor_reduce(xtg[:, b : b + 1], gB[:], axis=Ax.X, op=Alu.add)

            # final stats: u in wk/wkv; squares and cubes scratch in cand
            nc.scalar.activation(wk[:], cand[:, 0:HF], Act.Relu, bias=nu, scale=1.0)
            nc.vector.tensor_scalar(
                out=wkv[:, 0:VH], in0=cand[:, HF:CW], scalar1=th, scalar2=0.0,
                op0=Alu.subtract, op1=Alu.max,
            )
            nc.scalar.activation(cand[:, 0:HF], wk[:], Act.Square)
            nc.vector.tensor_tensor(out=cand[:, HF:CW], in0=wkv[:, 0:VH], in1=wkv[:, 0:VH], op=Alu.mult)
            nc.vector.tensor_tensor(out=cand[:, 0:HF], in0=cand[:, 0:HF], in1=wk[:], op=Alu.mult)
            nc.vector.tensor_reduce(S3fa, cand[:, 0:HF], axis=Ax.X, op=Alu.add)
            nc.vector.tensor_tensor(out=cand[:, HF:CW], in0=cand[:, HF:CW], in1=wkv[:, 0:VH], op=Alu.mult)
            nc.vector.tensor_reduce(S3fb, cand[:, HF:CW], axis=Ax.X, op=Alu.add)
            nc.vector.tensor_tensor(out=S3fa, in0=S3fa, in1=S3fb, op=Alu.add)
            # loss = 4/3 + S3f/12 + th - x_t   (S2f == 4 at the converged root)
            nc.vector.scalar_tensor_tensor(
                out=tb_, in0=S3fa, scalar=1.0 / 12.0, in1=th, op0=Alu.mult, op1=Alu.add
            )
            nc.vector.scalar_tensor_tensor(
                out=lo, in0=tb_, scalar=4.0 / 3.0, in1=xtg[:, b : b + 1],
                op0=Alu.add, op1=Alu.subtract,
            )
            nc.sync.dma_start(out[bass.ts(b, 128)], lo)

        phase_A(0)
        phase_BCDE(0)
        phase_A(1)
        phase_F(0)
        phase_BCDE(1)
        phase_F(1)

    nc.compile()
    return nc


def get_nc():
    if "nc" not in _nc_cache:
        _nc_cache["nc"] = _build_nc()
    return _nc_cache["nc"]


def make_in_maps(X, target):
    import ml_dtypes

    X = np.ascontiguousarray(np.asarray(X, dtype=np.float32))
    target = np.asarray(target).astype(np.int64)

    # wrapped gid+1 iota: iot[p, f] = 16*f + (p % 16) + 1
    pp, ff = np.meshgrid(np.arange(128), np.arange(WF), indexing="ij")
    iot = (16 * ff + (pp % 16) + 1).astype(np.float32)
    # residue-selection matrices: wsel[p, w, n] = 1 if n == 16*(p//16) + w
    wsel = np.zeros((128, 16, 128), np.float32)
    for w in range(16):
        for p in range(128):
            wsel[p, w, 16 * (p // 16) + w] = 1.0
    wsel = wsel.reshape(128, 16 * 128).astype(ml_dtypes.bfloat16)

    in_maps = []
    for k in range(N_CORES):
        Xk = X[k * ROWS : (k + 1) * ROWS]
        tk = target[k * ROWS : (k + 1) * ROWS]
        ohk = np.zeros((ROWS, 64), np.float32)
        ohk[np.arange(ROWS), (tk % 64).astype(np.int64)] = 1.0
        tblk = np.zeros((128, 16), np.int16)
        for c in range(4):
            rows = np.arange(64)
            vals = (rows * (V // 64) + (tk[64 * c + rows] // 64)).astype(np.int16)
            w = np.zeros((16, 4), np.int16)
            w[rows % 16, rows // 16] = vals
            tblk[:, 4 * c : 4 * (c + 1)] = np.tile(w, (8, 1))
        in_maps.append({"x": Xk, "oh": ohk, "tbl": tblk, "iot": iot, "wsel": wsel})
    return in_maps


def kernel(X, target):
    from concourse.bass_utils import run_bass_kernel_spmd

    nc = get_nc()
    in_maps = make_in_maps(X, target)
    res = run_bass_kernel_spmd(nc, in_maps, core_ids=list(range(N_CORES)))
    loss = np.concatenate([r["loss"] for r in res.results]).astype(np.float32)
    return loss
